# revision 1
# baseline (speedup 1.0000x reference)
"""TopK-SAE on 8 TRN2 cores — v3: two collective-free launches + host merge.

Launch 1 (dict-sharded): z0 = fp16 encode, per-dict-row top-8 values +
batch indices straight from PSUM -> per-core candidate tables. Pure
matmul/DVE/activation — the instruction mix of the proven baseline.
Host: merge candidate tables (per-shard top-k + merge), exact fp32
re-dots for the ~1k boundary candidates (fp16 noise band), exact global
top-K; builds per-core pre-gathered decode inputs (W_dec rows for the K
selected latents, A-sliced; batch ids; acts).
Launch 2 (A-sharded): x_hatT slice = G.T-contract @ one-hot(P) by
matmul; P built on DVE from an iota compare. No ReduceScatter.
"""
import os

import numpy as np

if os.environ.get("KV3_SMALL"):
    B, A, D, K = 256, 1024, 4096, 128
else:
    B, A, D, K = 2048, 4096, 32768, 4096
NCORES = 8
DL = D // NCORES
KT = A // 128
DT = DL // 128
AS = A // NCORES
KP = K                      # decode slots (K is a multiple of 128)
KG = KP // 128              # decode groups
BCH = min(512, B)
NBCH = B // BCH
DELTA = 0.004               # fp16 z0 noise band half-width (sigma ~3e-4)

_CACHE = {}


def build_enc():
    import concourse.bacc as bacc
    import concourse.mybir as mybir
    from concourse import tile

    f32 = mybir.dt.float32
    f16 = mybir.dt.float16
    u32 = mybir.dt.uint32
    Act = mybir.ActivationFunctionType

    nc = bacc.Bacc("TRN2", target_bir_lowering=False, debug=False,
                   num_devices=NCORES)
    xTh = nc.dram_tensor("xTh", [A, B], f16, kind="ExternalInput")
    wencTh = nc.dram_tensor("wencTh", [A, DL], f16, kind="ExternalInput")
    benc = nc.dram_tensor("benc", [DL, 1], f32, kind="ExternalInput")
    cand_v = nc.dram_tensor("cand_v", [128, DT * 8], f32,
                            kind="ExternalOutput")
    cand_i = nc.dram_tensor("cand_i", [128, DT * 8], u32,
                            kind="ExternalOutput")

    xTh_r = xTh.rearrange("(k p) c -> p k c", p=128)
    wencTh_r = wencTh.rearrange("(k p) c -> p k c", p=128)
    benc_r = benc.rearrange("(d p) c -> p (d c)", p=128)

    with tile.TileContext(nc) as tc:
        with (
            tc.tile_pool(name="uni", bufs=1) as unip,
            tc.tile_pool(name="big", bufs=1) as bigp,
            tc.tile_pool(name="wt", bufs=3) as wtp,
            tc.tile_pool(name="sm", bufs=2) as smp,
            tc.tile_pool(name="ps", bufs=2, space="PSUM") as pse,
        ):
            benc_sb = unip.tile([128, DT], f32, tag="benc", name="benc")
            nc.sync.dma_start(benc_sb[:], benc_r)
            cv = unip.tile([128, DT * 8], f32, tag="cv", name="cv")
            ci = unip.tile([128, DT * 8], u32, tag="ci", name="ci")
            xh = bigp.tile([128, KT * B], f16, tag="big", name="bigx")
            for q in range(4):
                kq = KT // 4
                nc.sync.dma_start(
                    xh[:, q * kq * B:(q + 1) * kq * B]
                    .rearrange("p (k c) -> p k c", c=B),
                    xTh_r[:, q * kq:(q + 1) * kq, :])
            for d in range(DT):
                wth = wtp.tile([128, KT * 128], f16, tag="wt", name="wt")
                nc.sync.dma_start(
                    wth[:].rearrange("p (k c) -> p k c", c=128),
                    wencTh_r[:, :, d * 128:(d + 1) * 128])
                zps = pse.tile([128, B], f32, tag="zps", name="zps")
                for n in range(NBCH):
                    for k in range(KT):
                        nc.tensor.matmul(
                            zps[:, n * BCH:(n + 1) * BCH],
                            wth[:, k * 128:(k + 1) * 128],
                            xh[:, k * B + n * BCH:k * B + (n + 1) * BCH],
                            start=(k == 0), stop=(k == KT - 1))
                mv = smp.tile([128, 8], f32, tag="mv", name="mv")
                nc.vector.max(mv[:], zps[:])
                nc.vector.max_index(ci[:, d * 8:(d + 1) * 8], mv[:], zps[:])
                nc.scalar.activation(cv[:, d * 8:(d + 1) * 8], mv[:],
                                     Act.Relu, bias=benc_sb[:, d:d + 1])
            nc.sync.dma_start(cand_v[:, :], cv[:])
            nc.sync.dma_start(cand_i[:, :], ci[:])
    nc.compile()
    return nc


def build_dec():
    import concourse.bacc as bacc
    import concourse.mybir as mybir
    from concourse import tile

    f32 = mybir.dt.float32
    f16 = mybir.dt.float16
    Alu = mybir.AluOpType

    nc = bacc.Bacc("TRN2", target_bir_lowering=False, debug=False,
                   num_devices=NCORES)
    Gin = nc.dram_tensor("Gin", [KP, AS], f16, kind="ExternalInput")
    bacts = nc.dram_tensor("bacts", [2, KP], f32, kind="ExternalInput")
    iotab_in = nc.dram_tensor("iotab_in", [128, B], f32,
                              kind="ExternalInput")
    out = nc.dram_tensor("out", [AS, B], f32, kind="ExternalOutput")

    Gin_r = Gin.rearrange("(g p) e -> p g e", p=128)       # [128, KG, AS]
    ba_r = [bacts[i:i + 1, :].rearrange("o (g p) -> p (o g)", p=128)
            for i in range(2)]                             # [128, KG] each

    with tile.TileContext(nc) as tc:
        with (
            tc.tile_pool(name="uni", bufs=1) as unip,
            tc.tile_pool(name="pg", bufs=3) as pgp,
            tc.tile_pool(name="sm", bufs=2) as smp,
            tc.tile_pool(name="ps", bufs=2, space="PSUM") as psd,
        ):
            iota_b = unip.tile([128, B], f32, tag="iob", name="iob")
            nc.sync.dma_start(iota_b[:], iotab_in[:, :])
            bA = unip.tile([128, KG], f32, tag="bA", name="bA")
            nc.sync.dma_start(bA[:], ba_r[0])
            vA = unip.tile([128, KG], f32, tag="vA", name="vA")
            nc.sync.dma_start(vA[:], ba_r[1])
            G = unip.tile([128, KG * AS], f16, tag="G", name="G")
            nc.sync.dma_start(
                G[:].rearrange("p (g e) -> p g e", e=AS), Gin_r)
            for n in range(NBCH):
                dpss = [psd.tile([128, BCH], f32, tag=f"dps{at}",
                                 name=f"dps{at}")
                        for at in range(AS // 128)]
                for g in range(KG):
                    pg_t = pgp.tile([128, BCH], f16, tag="pgt", name="pgt")
                    pg = pgp.tile([128, BCH], f16, tag="pg", name="pg")
                    nc.vector.scalar_tensor_tensor(
                        pg_t[:], iota_b[:, n * BCH:(n + 1) * BCH],
                        bA[:, g:g + 1], iota_b[:, n * BCH:(n + 1) * BCH],
                        Alu.is_equal, Alu.bypass)
                    nc.vector.scalar_tensor_tensor(
                        pg[:], pg_t[:], vA[:, g:g + 1], pg_t[:],
                        Alu.mult, Alu.bypass)
                    for at in range(AS // 128):
                        nc.tensor.matmul(
                            dpss[at][:],
                            G[:, g * AS + at * 128:g * AS + at * 128 + 128],
                            pg[:], start=(g == 0), stop=(g == KG - 1))
                for at in range(AS // 128):
                    osb = smp.tile([128, BCH], f32, tag="osb", name="osb")
                    nc.vector.tensor_copy(osb[:], dpss[at][:])
                    nc.sync.dma_start(
                        out[at * 128:(at + 1) * 128,
                            n * BCH:(n + 1) * BCH], osb[:])
    nc.compile()
    return nc


def _get_ncs():
    if "enc" not in _CACHE:
        _CACHE["enc"] = build_enc()
        _CACHE["dec"] = build_dec()
    return _CACHE["enc"], _CACHE["dec"]


def kernel(x, W_enc, b_enc, W_dec, b_dec):
    from concourse.bass_utils import run_bass_kernel_spmd

    x = np.asarray(x, np.float32)
    W_enc = np.asarray(W_enc, np.float32)
    b_enc = np.asarray(b_enc, np.float32)
    W_dec = np.asarray(W_dec, np.float32)
    b_dec = np.asarray(b_dec, np.float32)
    nc_enc, nc_dec = _get_ncs()

    xa = x - b_dec[None, :]
    xTh = np.ascontiguousarray(xa.astype(np.float16).T)
    in1 = []
    for i in range(NCORES):
        sl = slice(i * DL, (i + 1) * DL)
        in1.append({
            "xTh": xTh,
            "wencTh": np.ascontiguousarray(W_enc[sl].T.astype(np.float16)),
            "benc": np.ascontiguousarray(b_enc[sl]).reshape(DL, 1),
        })
    r1 = run_bass_kernel_spmd(nc_enc, in1, core_ids=list(range(NCORES)))

    # ---- host merge: per-shard top-8 candidates -> exact global top-K ----
    dloc = (np.arange(128)[:, None]
            + 128 * (np.arange(DT * 8)[None, :] // 8))
    cv = np.stack([r1.results[c]["cand_v"] for c in range(NCORES)])
    bi = np.stack([r1.results[c]["cand_i"].astype(np.int64)
                   for c in range(NCORES)])
    dg = (dloc[None, :, :] + (np.arange(NCORES) * DL)[:, None, None])
    cvf, bif, dgf = cv.ravel(), bi.ravel(), dg.ravel()
    # rough threshold from z0 candidates, then exact re-dot of the band
    kth0 = np.partition(cvf, -K)[-K]
    band = np.abs(cvf - kth0) <= DELTA
    definite = cvf > kth0 + DELTA
    if band.any():
        bd_idx = np.nonzero(band)[0]
        zex = np.einsum("ij,ij->i", W_enc[dgf[bd_idx]], xa[bif[bd_idx]],
                        optimize=True) + b_enc[dgf[bd_idx]]
        cvf = cvf.copy()
        cvf[bd_idx] = zex
    need = K - int(definite.sum())
    bsel = np.zeros_like(band)
    if band.any() and need > 0:
        bv = cvf[bd_idx]
        order = np.argsort(-bv)[:need]
        bsel[bd_idx[order]] = True
    sel = np.nonzero(definite | bsel)[0]
    sel = sel[np.argsort(-cvf[sel])][:K]
    acts = cvf[sel]
    rows_b = bif[sel]
    cols_d = dgf[sel]
    npad = K - len(sel)
    if npad:
        acts = np.pad(acts, (0, npad))
        rows_b = np.pad(rows_b, (0, npad))
        cols_d = np.pad(cols_d, (0, npad))

    # ---- decode inputs: pre-gathered W_dec rows, A-sliced per core ----
    Wsel = W_dec[cols_d].astype(np.float16)                # [K, A]
    bacts = np.stack([rows_b.astype(np.float32), acts.astype(np.float32)])
    iotab = np.broadcast_to(np.arange(B, dtype=np.float32)[None, :],
                            (128, B)).copy()
    in2 = [{"Gin": np.ascontiguousarray(Wsel[:, c * AS:(c + 1) * AS]),
            "bacts": bacts, "iotab_in": iotab} for c in range(NCORES)]
    r2 = run_bass_kernel_spmd(nc_dec, in2, core_ids=list(range(NCORES)))

    xhatT = np.empty((A, B), np.float32)
    for c in range(NCORES):
        xhatT[c * AS:(c + 1) * AS, :] = r2.results[c]["out"]
    return np.ascontiguousarray(xhatT.T) + b_dec[None, :]



# revision 2
# speedup vs baseline: 1.9805x; 1.9805x over previous
"""TopK-SAE on 8 TRN2 cores — v4: fp8 DoubleRow encode + chunked decode.

Launch 1 (dict-sharded): z0 = fp8e4 DoubleRow encode (W pre-scaled by 64,
fp32 PSUM accumulation), per-dict-row top-8 values + batch indices from
PSUM -> per-core candidate tables. 2x PE throughput vs the fp16 baseline.
Host: exact merge — union of all candidates with noisy value >= kth0-DELTA
re-dotted (fp32 einsum + fp64 re-dot inside a tight boundary window),
exact global top-K; latents sorted by batch row and packed into per-
batch-chunk slot groups (NG groups of 128 per 512-row chunk).
Launch 2 (A-sharded): x_hatT slice = G.T @ one-hot(P) accumulated only
over the NG groups belonging to each batch chunk (~2.7x fewer decode
flops than the global-slot baseline).
"""
import numpy as np

B, A, D, K = 2048, 4096, 32768, 4096
NCORES = 8
DL = D // NCORES            # dict rows per core
DT = DL // 128              # d-tiles per core
KT2 = A // 256              # DoubleRow contraction tiles
BCH = 512
NBCH = B // BCH
NG = 12                     # decode slot groups per batch chunk
SLOTC = NG * 128            # slot capacity per chunk
KP2 = NBCH * SLOTC          # total decode slots
AS = A // NCORES            # A-shard per core
AT = AS // 128
WSCALE = 64.0
DELTA = 0.25                # fp8 z0 noise band (sigma ~0.0375, max ~0.21)
FP64_WND = 0.002            # fp64 re-dot window around the boundary

_CACHE = {}


def build_enc():
    import concourse.bacc as bacc
    import concourse.mybir as mybir
    from concourse import tile

    f32 = mybir.dt.float32
    f8 = mybir.dt.float8e4
    u32 = mybir.dt.uint32
    Act = mybir.ActivationFunctionType
    DR = mybir.MatmulPerfMode.DoubleRow

    nc = bacc.Bacc("TRN2", target_bir_lowering=False, debug=False,
                   num_devices=NCORES)
    xp = nc.dram_tensor("xp", [NBCH, 128, KT2 * 2 * BCH], f8,
                        kind="ExternalInput")
    wp = nc.dram_tensor("wp", [DT, 128, KT2 * 2 * 128], f8,
                        kind="ExternalInput")
    benc = nc.dram_tensor("benc", [DL, 1], f32, kind="ExternalInput")
    cand_v = nc.dram_tensor("cand_v", [128, DT * 8], f32,
                            kind="ExternalOutput")
    cand_i = nc.dram_tensor("cand_i", [128, DT * 8], u32,
                            kind="ExternalOutput")

    benc_r = benc.rearrange("(d p) c -> p (d c)", p=128)

    with tile.TileContext(nc) as tc:
        with (
            tc.tile_pool(name="uni", bufs=1) as unip,
            tc.tile_pool(name="wt", bufs=4) as wtp,
            tc.tile_pool(name="sm", bufs=2) as smp,
            tc.tile_pool(name="ps", bufs=2, space="PSUM") as pse,
        ):
            benc_sb = unip.tile([128, DT], f32, tag="benc", name="benc")
            nc.sync.dma_start(benc_sb[:], benc_r)
            cv = unip.tile([128, DT * 8], f32, tag="cv", name="cv")
            ci = unip.tile([128, DT * 8], u32, tag="ci", name="ci")
            xvs = []
            for n in range(NBCH):
                xt = unip.tile([128, KT2 * 2 * BCH], f8, tag=f"x{n}",
                               name=f"x{n}")
                nc.sync.dma_start(xt[:], xp[n, :, :])
                xvs.append(xt[:].rearrange("p (kt ko c) -> p kt ko c",
                                           ko=2, c=BCH))
            for d in range(DT):
                wth = wtp.tile([128, KT2 * 2 * 128], f8, tag="wt", name="wt")
                nc.sync.dma_start(wth[:], wp[d, :, :])
                wv = wth[:].rearrange("p (kt ko m) -> p kt ko m",
                                      ko=2, m=128)
                zps = pse.tile([128, B], f32, tag="zps", name="zps")
                for kt in range(KT2):
                    for n in range(NBCH):
                        nc.tensor.matmul(
                            zps[:, n * BCH:(n + 1) * BCH],
                            wv[:, kt], xvs[n][:, kt],
                            start=(kt == 0), stop=(kt == KT2 - 1),
                            perf_mode=DR)
                mv = smp.tile([128, 8], f32, tag="mv", name="mv")
                nc.vector.max(mv[:], zps[:])
                nc.vector.max_index(ci[:, d * 8:(d + 1) * 8], mv[:], zps[:])
                nc.scalar.activation(cv[:, d * 8:(d + 1) * 8], mv[:],
                                     Act.Relu, bias=benc_sb[:, d:d + 1],
                                     scale=1.0 / WSCALE)
            nc.sync.dma_start(cand_v[:, :], cv[:])
            nc.sync.dma_start(cand_i[:, :], ci[:])
    nc.compile()
    return nc


def build_dec():
    import concourse.bacc as bacc
    import concourse.mybir as mybir
    from concourse import tile

    f32 = mybir.dt.float32
    f16 = mybir.dt.float16
    Alu = mybir.AluOpType

    nc = bacc.Bacc("TRN2", target_bir_lowering=False, debug=False,
                   num_devices=NCORES)
    Gin = nc.dram_tensor("Gin", [KP2, AS], f16, kind="ExternalInput")
    bacts = nc.dram_tensor("bacts", [2, KP2], f32, kind="ExternalInput")
    iotab_in = nc.dram_tensor("iotab_in", [128, B], f32,
                              kind="ExternalInput")
    out = nc.dram_tensor("out", [AS, B], f32, kind="ExternalOutput")

    Gin_r = Gin.rearrange("(g p) e -> p g e", p=128)     # [128, KP2/128, AS]
    ba_r = [bacts[i:i + 1, :].rearrange("o (g p) -> p (o g)", p=128)
            for i in range(2)]                           # [128, KP2/128]

    with tile.TileContext(nc) as tc:
        with (
            tc.tile_pool(name="uni", bufs=1) as unip,
            tc.tile_pool(name="gg", bufs=2) as ggp,
            tc.tile_pool(name="pg", bufs=3) as pgp,
            tc.tile_pool(name="sm", bufs=2) as smp,
            tc.tile_pool(name="ps", bufs=2, space="PSUM") as psd,
        ):
            iota_b = unip.tile([128, B], f32, tag="iob", name="iob")
            nc.sync.dma_start(iota_b[:], iotab_in[:, :])
            bA = unip.tile([128, KP2 // 128], f32, tag="bA", name="bA")
            nc.sync.dma_start(bA[:], ba_r[0])
            vA = unip.tile([128, KP2 // 128], f32, tag="vA", name="vA")
            nc.sync.dma_start(vA[:], ba_r[1])
            for n in range(NBCH):
                G = ggp.tile([128, NG * AS], f16, tag="G", name="G")
                nc.sync.dma_start(
                    G[:].rearrange("p (g e) -> p g e", e=AS),
                    Gin_r[:, n * NG:(n + 1) * NG, :])
                dpss = [psd.tile([128, BCH], f32, tag=f"dps{at}",
                                 name=f"dps{at}")
                        for at in range(AT)]
                for g in range(NG):
                    ggi = n * NG + g
                    pg_t = pgp.tile([128, BCH], f16, tag="pgt", name="pgt")
                    pg = pgp.tile([128, BCH], f16, tag="pg", name="pg")
                    nc.vector.scalar_tensor_tensor(
                        pg_t[:], iota_b[:, n * BCH:(n + 1) * BCH],
                        bA[:, ggi:ggi + 1],
                        iota_b[:, n * BCH:(n + 1) * BCH],
                        Alu.is_equal, Alu.bypass)
                    nc.vector.scalar_tensor_tensor(
                        pg[:], pg_t[:], vA[:, ggi:ggi + 1], pg_t[:],
                        Alu.mult, Alu.bypass)
                    for at in range(AT):
                        nc.tensor.matmul(
                            dpss[at][:],
                            G[:, g * AS + at * 128:g * AS + at * 128 + 128],
                            pg[:], start=(g == 0), stop=(g == NG - 1))
                for at in range(AT):
                    osb = smp.tile([128, BCH], f32, tag="osb", name="osb")
                    nc.vector.tensor_copy(osb[:], dpss[at][:])
                    nc.sync.dma_start(
                        out[at * 128:(at + 1) * 128,
                            n * BCH:(n + 1) * BCH], osb[:])
    nc.compile()
    return nc


def _get_ncs():
    if "enc" not in _CACHE:
        _CACHE["enc"] = build_enc()
        _CACHE["dec"] = build_dec()
    return _CACHE["enc"], _CACHE["dec"]


def _pack_x(xa):
    import ml_dtypes
    x8t = np.ascontiguousarray(xa.T).astype(ml_dtypes.float8_e4m3)  # [A, B]
    arr = x8t.reshape(KT2, 2, 128, B).transpose(2, 0, 1, 3)  # [128,kt,ko,B]
    return [np.ascontiguousarray(
        arr[:, :, :, n * BCH:(n + 1) * BCH]).reshape(128, KT2 * 2 * BCH)
        for n in range(NBCH)]


def _pack_w(Wc):
    """Wc: [DL, A] fp32 core shard -> [DT, 128, KT2*2*128] fp8 (x64)."""
    import ml_dtypes
    w8t = np.ascontiguousarray(Wc.T * WSCALE).astype(ml_dtypes.float8_e4m3)
    arr = w8t.reshape(KT2, 2, 128, DT, 128).transpose(3, 2, 0, 1, 4)
    return np.ascontiguousarray(arr).reshape(DT, 128, KT2 * 2 * 128)


def kernel(x, W_enc, b_enc, W_dec, b_dec):
    from concourse.bass_utils import run_bass_kernel_spmd

    x = np.asarray(x, np.float32)
    W_enc = np.asarray(W_enc, np.float32)
    b_enc = np.asarray(b_enc, np.float32)
    W_dec = np.asarray(W_dec, np.float32)
    b_dec = np.asarray(b_dec, np.float32)
    nc_enc, nc_dec = _get_ncs()

    xa = x - b_dec[None, :]
    xps = _pack_x(xa)
    in1 = []
    for i in range(NCORES):
        sl = slice(i * DL, (i + 1) * DL)
        in1.append({
            "xp": np.stack(xps),
            "wp": _pack_w(W_enc[sl]),
            "benc": np.ascontiguousarray(b_enc[sl]).reshape(DL, 1),
        })
    r1 = run_bass_kernel_spmd(nc_enc, in1, core_ids=list(range(NCORES)))

    # ---- host merge: per-dict-row top-8 candidates -> exact global top-K --
    dloc = (np.arange(128)[:, None]
            + 128 * (np.arange(DT * 8)[None, :] // 8))
    cv = np.stack([r1.results[c]["cand_v"] for c in range(NCORES)])
    bi = np.stack([r1.results[c]["cand_i"].astype(np.int64)
                   for c in range(NCORES)])
    dg = (dloc[None, :, :] + (np.arange(NCORES) * DL)[:, None, None])
    cvf, bif, dgf = cv.ravel(), bi.ravel(), dg.ravel()
    kth0 = np.partition(cvf, -K)[-K]
    uni = np.nonzero(cvf >= kth0 - DELTA)[0]
    ub, ud = bif[uni], dgf[uni]
    v32 = (np.einsum("ij,ij->i", W_enc[ud], xa[ub], optimize=True)
           + b_enc[ud])
    kth32 = np.partition(v32, -K)[-K]
    wnd = np.abs(v32 - kth32) <= FP64_WND
    if wnd.any():
        wi = np.nonzero(wnd)[0]
        v32 = v32.astype(np.float64)
        v32[wi] = (np.einsum("ij,ij->i", W_enc[ud[wi]].astype(np.float64),
                             xa[ub[wi]].astype(np.float64))
                   + b_enc[ud[wi]])
    order = np.argsort(-v32)[:K]
    acts = np.maximum(v32[order], 0.0).astype(np.float32)
    rows_b = ub[order]
    cols_d = ud[order]

    # ---- sort by batch row, pack into per-chunk slot groups ----
    srt = np.argsort(rows_b, kind="stable")
    acts, rows_b, cols_d = acts[srt], rows_b[srt], cols_d[srt]
    chunk = rows_b // BCH
    s_acts = np.zeros(KP2, np.float32)
    s_rows = np.full(KP2, -1.0, np.float32)
    s_cols = np.zeros(KP2, np.int64)
    for n in range(NBCH):
        idx = np.nonzero(chunk == n)[0]
        if len(idx) > SLOTC:
            # astronomically unlikely; keep the largest acts
            keep = np.argsort(-acts[idx])[:SLOTC]
            idx = idx[np.sort(keep)]
        base = n * SLOTC
        s_acts[base:base + len(idx)] = acts[idx]
        s_rows[base:base + len(idx)] = rows_b[idx].astype(np.float32)
        s_cols[base:base + len(idx)] = cols_d[idx]

    Wsel = W_dec[s_cols].astype(np.float16)                # [KP2, A]
    bacts = np.stack([s_rows, s_acts])
    iotab = np.broadcast_to(np.arange(B, dtype=np.float32)[None, :],
                            (128, B)).copy()
    in2 = [{"Gin": np.ascontiguousarray(Wsel[:, c * AS:(c + 1) * AS]),
            "bacts": bacts, "iotab_in": iotab} for c in range(NCORES)]
    r2 = run_bass_kernel_spmd(nc_dec, in2, core_ids=list(range(NCORES)))

    xhatT = np.empty((A, B), np.float32)
    for c in range(NCORES):
        xhatT[c * AS:(c + 1) * AS, :] = r2.results[c]["out"]
    return np.ascontiguousarray(xhatT.T) + b_dec[None, :]
